# revision 7
# baseline (speedup 1.0000x reference)
"""Trainium2 Bass kernel for KroneckerLinear: y = x @ kron(U, V).

Math: with x[t] reshaped to X_t [i1=128, i2=128] (i2 contiguous) and
y[t] reshaped to Y_t [j1=128, j2=128] (j2 contiguous):

    Y_t = U^T @ X_t @ V

Dataflow (all bf16 on the wire, fp32 accumulation in PSUM):

  stage 1 (per token, token tile stationary):
      MM: out = lhsT.T @ rhs, lhsT = X_t [i1, i2], rhs = U [i1, j1]
          -> P_t = X_t^T U  laid out [i2, j1] in PSUM.
      Four tokens share one PSUM bank: pa = [i2, (t4, j1)].
  stage 2 (batched, V stationary, loaded once):
      MM: lhsT = V [i2, j2], rhs = P [i2, (t4, j1)] (bf16 copy of pa)
          -> Y [j2, (t4, j1)] in PSUM, one N=512 matmul per 4 tokens.

Layouts are chosen so every DMA is a contiguous per-partition run:
  x is pre-swizzled on the host to [i1, t, i2] (per core), y leaves the
  device as [j2, t, j1] and is unscrambled on the host. With group=32
  tokens per DMA that is 8 KB per partition per transfer.

Sharding: data-parallel over tokens, 256 tokens per core x 8 cores.
Host converts x/U/V to bf16 (rel err ~5e-3 vs fp32 reference) and
upcasts the bf16 y back to fp32.
"""

import sys

if "/opt/trn_rl_repo" not in sys.path:
    sys.path.insert(0, "/opt/trn_rl_repo")

import ml_dtypes
import numpy as np

import concourse.bacc as bacc
import concourse.bass as bass
import concourse.mybir as mybir
from concourse import tile
from concourse.bass_utils import run_bass_kernel_spmd

F32 = mybir.dt.float32
BF16 = mybir.dt.bfloat16
NP_BF16 = ml_dtypes.bfloat16

N_CORES = 8
TOKENS = 2048
D = 16384  # 128 * 128
T_CORE = TOKENS // N_CORES  # 256


def build_nc(n_tokens=T_CORE, group=32, scheme="alt"):
    """Build + compile the per-core program.

    group: tokens per load/store DMA (and per x/y SBUF tile).
    scheme: engine assignment for the two PSUM->SBUF copies per 8-token
    oct — "alt" alternates P/Y between vector and scalar per oct,
    "va" puts P on vector and Y on scalar.
    """
    assert n_tokens % group == 0 and group % 8 == 0

    nc = bacc.Bacc("TRN2", target_bir_lowering=False, debug=False)
    x = nc.dram_tensor("x", [128, n_tokens * 128], BF16, kind="ExternalInput")
    u = nc.dram_tensor("u", [128, 128], BF16, kind="ExternalInput")
    v = nc.dram_tensor("v", [128, 128], BF16, kind="ExternalInput")
    y = nc.dram_tensor("y", [128, n_tokens * 128], BF16, kind="ExternalOutput")

    vcopy = nc.vector.tensor_copy
    scopy = nc.scalar.copy

    with tile.TileContext(nc) as tc:
        with (
            tc.tile_pool(name="const", bufs=1) as cpool,
            tc.tile_pool(name="xin", bufs=6) as xpool,
            tc.tile_pool(name="yout", bufs=6) as ypool,
            tc.tile_pool(name="pmid", bufs=3) as ppool,
            tc.tile_pool(name="psa", bufs=2, space="PSUM") as pspool_a,
            tc.tile_pool(name="psb", bufs=2, space="PSUM") as pspool_b,
        ):
            u_sb = cpool.tile([128, 128], BF16)
            v_sb = cpool.tile([128, 128], BF16)

            nc.sync.dma_start(u_sb[:], u[:])
            nc.sync.dma_start(v_sb[:], v[:])

            gcols = group * 128
            n_groups = n_tokens // group
            octs = group // 8
            for g in range(n_groups):
                xt = xpool.tile([128, gcols], BF16)
                nc.sync.dma_start(xt[:], x[:, g * gcols : (g + 1) * gcols])
                for o in range(octs):
                    pa = pspool_a.tile([128, 1024], F32)
                    for k in range(8):
                        t = (o * 8 + k) * 128
                        nc.tensor.matmul(
                            pa[:, k * 128 : (k + 1) * 128],
                            lhsT=xt[:, t : t + 128],
                            rhs=u_sb[:],
                            start=True,
                            stop=True,
                        )
                    ps = ppool.tile([128, 1024], BF16)
                    vcopy(ps[:], pa[:])
                    pb = pspool_b.tile([128, 1024], F32)
                    nc.tensor.matmul(
                        pb[:, 0:512], lhsT=v_sb[:], rhs=ps[:, 0:512],
                        start=True, stop=True,
                    )
                    nc.tensor.matmul(
                        pb[:, 512:1024], lhsT=v_sb[:], rhs=ps[:, 512:1024],
                        start=True, stop=True,
                    )
                    yt = ypool.tile([128, 1024], BF16)
                    scopy(yt[:], pb[:])
                    c0 = (g * octs + o) * 1024
                    nc.gpsimd.dma_start(y[:, c0 : c0 + 1024], yt[:])
    nc.compile()
    return nc


_NC_CACHE = {}


def _get_nc(n_tokens, group, scheme):
    key = (n_tokens, group, scheme)
    if key not in _NC_CACHE:
        _NC_CACHE[key] = build_nc(n_tokens, group, scheme)
    return _NC_CACHE[key]


def run(x, U, V, group=32, scheme="alt", trace=False, **spmd_kwargs):
    """Shard over 8 cores, run, gather. Returns (y_full, BassKernelResults)."""
    x = np.ascontiguousarray(np.asarray(x), dtype=np.float32)
    U = np.ascontiguousarray(np.asarray(U), dtype=np.float32).astype(NP_BF16)
    V = np.ascontiguousarray(np.asarray(V), dtype=np.float32).astype(NP_BF16)
    t_total = x.shape[0]
    t_core = t_total // N_CORES
    xb = x.astype(NP_BF16)

    nc = _get_nc(t_core, group, scheme)
    in_maps = []
    for c in range(N_CORES):
        xc = xb[c * t_core : (c + 1) * t_core].reshape(t_core, 128, 128)
        xc = np.ascontiguousarray(xc.transpose(1, 0, 2)).reshape(128, t_core * 128)
        in_maps.append({"x": xc, "u": U, "v": V})
    res = run_bass_kernel_spmd(
        nc, in_maps, list(range(N_CORES)), trace=trace, **spmd_kwargs
    )
    out = np.empty((t_total, D), dtype=np.float32)
    for c in range(N_CORES):
        yc = np.asarray(res.results[c]["y"]).reshape(128, t_core, 128)
        # [j2, t, j1] -> [t, j1, j2]
        out[c * t_core : (c + 1) * t_core] = (
            yc.transpose(1, 2, 0).reshape(t_core, D).astype(np.float32)
        )
    return out, res


def kernel(x, U, V):
    out, _ = run(x, U, V)
    return out
